# revision 17
# baseline (speedup 1.0000x reference)
"""Causal self-attention (B=2, T=2048, C=1024, H=16, Dh=64) on 8 trn2 NeuronCores.

Sharding: 2-way data-parallel over batch x 4-way tensor-parallel over heads.
Core c handles batch b=c//4 and heads 4g..4g+3 where g=c%4; it returns a
partial [T, C] row-projection which the host sums over the 4 head groups.

v2 schedule (vs the 207us baseline):
- DMA-paced start: x and the qkv weights stream on two rings while the
  first qk projection wave runs k-OUTER across 8 PSUM banks, so the PE
  starts ~1.5us in instead of waiting for the full 4MB of x.
- Chunk-major attention: outer loop over q-chunks, then heads, then
  k-tiles. Only one PV accumulator bank is live at a time, leaving PSUM
  room to double-buffer scores, and the output projection (+ HBM
  writeback) for chunk j fires as soon as its 4 heads finish - output DMA
  is spread across the whole attention span instead of a 13us tail.
- No mask matmuls: the causal triangle of each diagonal block is zeroed
  POST-exp by a gpsimd affine_select (idle engine), saving ~20k PE
  columns and 64 weight loads.
- PV lags scores by one k-tile so the scalar-engine exp latency never
  stalls the PE; independent GEMM work (V projection, second qk wave,
  output projection) is interleaved into the attention loops as filler.
- Epilogue per (chunk, head): softmax denominator comes free as a ones
  column in the PV stationary; 1/l via reciprocal_approx_fast (DVE) +
  gpsimd partition broadcast, fused into the PSUM->SBUF normalize mul.
"""

import numpy as np
import ml_dtypes
from contextlib import ExitStack

import concourse.bass as bass
import concourse.tile as tile
from concourse import bacc, mybir, bass_utils

F32 = mybir.dt.float32
BF16 = mybir.dt.bfloat16
FP8 = mybir.dt.float8e4

T = 2048
C = 1024
HL = 4   # local heads per core
DH = 64
NKT = T // 128   # 16 k-tiles
NQ = T // 512    # 4 q-chunks
NCC = C // 128   # 8 contraction chunks


def _pin_act_table():
    import concourse.bacc as bacc_mod
    from concourse.hw_specs import get_activation_tables as real

    def only_combined(arch):
        t = real(arch)
        name = "natural_log_exp_and_others"
        if name in t:
            return {name: t[name]}
        return t

    bacc_mod.get_activation_tables = only_combined


def build_nc():
    _pin_act_table()
    nc = bacc.Bacc("TRN2", target_bir_lowering=False, debug=False)
    xt_d = nc.dram_tensor("xt", [C, T], BF16, kind="ExternalInput").ap()
    wqkt_d = nc.dram_tensor("wqkt", [C, 512], BF16, kind="ExternalInput").ap()
    wvt_d = nc.dram_tensor("wvt", [C, 256], BF16, kind="ExternalInput").ap()
    wpt_d = nc.dram_tensor("wpt", [256, C], BF16, kind="ExternalInput").ap()
    p_d = nc.dram_tensor("p", [T, C], F32, kind="ExternalOutput").ap()

    with tile.TileContext(nc) as tc:
        with ExitStack() as ctx:
            _body(ctx, tc, xt_d, wqkt_d, wvt_d, wpt_d, p_d)
    nc.compile()
    return nc


def _body(ctx, tc, xt_d, wqkt_d, wvt_d, wpt_d, p_d):
    nc = tc.nc
    Exp = mybir.ActivationFunctionType.Exp

    persist = ctx.enter_context(tc.tile_pool(name="persist", bufs=1))
    ptp = ctx.enter_context(tc.tile_pool(name="ptp", bufs=5))
    rrp = ctx.enter_context(tc.tile_pool(name="rrp", bufs=4))
    tmpn = ctx.enter_context(tc.tile_pool(name="tmpn", bufs=2))
    pout = ctx.enter_context(tc.tile_pool(name="pout", bufs=4))
    # PSUM (8 banks): "st" ring of 2 x [128,2,512] (paired score tiles,
    # 4 banks), "ot" ring of 2 (PV accumulators), "wa" ring of 2 (filler
    # GEMM units: V / second qk wave / projection). 4+2+2 = 8.
    pp = ctx.enter_context(tc.tile_pool(name="pp", bufs=1, space="PSUM"))

    # ---- persistent SBUF tiles ----
    onesf = persist.tile([128, 64], F32, tag="onesf")
    xT = persist.tile([128, NCC, T], BF16, tag="xT")
    wqkT = persist.tile([128, NCC, 512], BF16, tag="wqkT")
    wvT = persist.tile([128, NCC, 256], BF16, tag="wvT")
    wpT = persist.tile([128, 2, C], BF16, tag="wpT")
    qkT = [persist.tile([128, T], FP8, tag=f"qkT{m}", name=f"qkT{m}")
           for m in range(4)]
    qk8 = [persist.tile([32, 2, 2, T], FP8, tag=f"qk8_{m}", name=f"qk8_{m}")
           for m in range(4)]
    vs = [persist.tile([128, HL, 128], BF16, tag=f"vs{i}", name=f"vs{i}")
          for i in range(NKT)]
    otj = [persist.tile([128, 2, 512], BF16, tag=f"otj{j}", name=f"otj{j}")
           for j in range(NQ)]
    identb = persist.tile([128, 128], BF16, tag="identb")
    maskb = persist.tile([128, 128], BF16, tag="maskb")

    nc.gpsimd.memset(onesf[:], 1.0)
    # bf16 identity (for PSUM-accumulate mask adds) and the causal triangle
    # mask: maskb[k, q] = 0 where q >= k else -30000 (additive, pre-exp).
    nc.gpsimd.memset(identb[:], 0.0)
    nc.gpsimd.affine_select(
        out=identb[:], in_=identb[:], compare_op=mybir.AluOpType.not_equal,
        fill=1.0, base=0, channel_multiplier=1, pattern=[[-1, 128]])
    nc.gpsimd.memset(maskb[:], 0.0)
    nc.gpsimd.affine_select(
        out=maskb[:], in_=maskb[:], compare_op=mybir.AluOpType.is_ge,
        fill=-30000.0, base=0, channel_multiplier=-1, pattern=[[1, 128]])
    for i in range(NKT):
        # col 64 of each head strip = softmax denominator ones; 65..127 pad.
        nc.vector.tensor_copy(
            vs[i][:, :, 64:128],
            onesf[:, 0:64].rearrange("p (a b) -> p a b", a=1).to_broadcast(
                (128, HL, 64)))

    # ---- input DMA: x split across sync+scalar rings, weights on gpsimd ----
    for k in range(NCC):
        eng = nc.sync if k % 2 == 0 else nc.scalar
        eng.dma_start(xT[:, k, :], xt_d[k * 128:(k + 1) * 128, :])
    for k in range(NCC):
        nc.gpsimd.dma_start(wqkT[:, k, :], wqkt_d[k * 128:(k + 1) * 128, :])
    for k in range(NCC):
        nc.gpsimd.dma_start(wvT[:, k, :], wvt_d[k * 128:(k + 1) * 128, :])
    for c in range(2):
        nc.gpsimd.dma_start(wpT[:, c, :], wpt_d[c * 128:(c + 1) * 128, :])

    # ---- joint qk wave m=0,2 (heads 0,1): k-outer across all 8 banks,
    # paced by x-chunk arrival so the PE starts ~1.5us in ----
    stA = [pp.tile([128, 2, 512], F32, tag="st", bufs=2, name=f"stA{t}")
           for t in range(2)]
    otA = [pp.tile([128, 512], F32, tag="ot", bufs=2, name=f"otA{t}")
           for t in range(2)]
    waA = [pp.tile([128, 512], F32, tag="wa", bufs=2, name=f"waA{t}")
           for t in range(2)]
    slotA = {}
    for n in range(NQ):
        slotA[(0, n)] = stA[n // 2][:, n % 2, :]
    slotA[(2, 0)] = otA[0][:]
    slotA[(2, 1)] = otA[1][:]
    slotA[(2, 2)] = waA[0][:]
    slotA[(2, 3)] = waA[1][:]
    for k in range(NCC):
        for m in (0, 2):
            for n in range(NQ):
                nc.tensor.matmul(
                    slotA[(m, n)],
                    lhsT=wqkT[:, k, m * 128:(m + 1) * 128],
                    rhs=xT[:, k, n * 512:(n + 1) * 512],
                    start=(k == 0), stop=(k == NCC - 1))
    def shuffle_qk(m):
        # [128(e s r), T] -> [32(r), e, s, T] for DoubleRow score matmuls
        for e in range(2):
            for s in range(2):
                base = e * 64 + s * 32
                eng = nc.sync if (e + s) % 2 == 0 else nc.gpsimd
                eng.dma_start(qk8[m][:, e, s, :],
                              qkT[m][base:base + 32, :])

    for m in (0, 2):
        for n in range(NQ):
            if n % 2 == 0:
                nc.scalar.copy(qkT[m][:, n * 512:(n + 1) * 512], slotA[(m, n)])
            else:
                nc.vector.tensor_copy(qkT[m][:, n * 512:(n + 1) * 512],
                                      slotA[(m, n)])
    for m in (0, 2):
        shuffle_qk(m)

    # ---- filler GEMM units (one "wa" bank each) ----
    def emit_A2_unit(m, n):  # one (m,n) block of the second qk wave
        ps = pp.tile([128, 512], F32, tag="wa", bufs=2)
        for k in range(NCC):
            nc.tensor.matmul(
                ps[:],
                lhsT=wqkT[:, k, m * 128:(m + 1) * 128],
                rhs=xT[:, k, n * 512:(n + 1) * 512],
                start=(k == 0), stop=(k == NCC - 1))
        nc.vector.tensor_copy(qkT[m][:, n * 512:(n + 1) * 512], ps[:])
        if n == NQ - 1:
            shuffle_qk(m)

    def emit_V_unit(i):  # v tile i
        ps = pp.tile([128, 256], F32, tag="wa", bufs=2)
        for k in range(NCC):
            nc.tensor.matmul(
                ps[:],
                lhsT=xT[:, k, i * 128:(i + 1) * 128],
                rhs=wvT[:, k, :],
                start=(k == 0), stop=(k == NCC - 1))
        nc.vector.tensor_copy(
            vs[i][:, :, 0:64], ps[:].rearrange("p (h d) -> p h d", h=HL))

    def emit_proj_unit(j, tbl):  # one 128-row block of chunk j's projection
        tb = 4 * j + tbl
        for n2 in range(2):
            ps = pp.tile([128, 512], F32, tag="wa", bufs=2)
            for c in range(2):
                nc.tensor.matmul(
                    ps[:],
                    lhsT=otj[j][:, c, tbl * 128:(tbl + 1) * 128],
                    rhs=wpT[:, c, n2 * 512:(n2 + 1) * 512],
                    start=(c == 0), stop=(c == 1))
            po = pout.tile([128, 512], F32, tag="po")
            nc.vector.tensor_copy(po[:], ps[:])
            nc.sync.dma_start(
                p_d[tb * 128:(tb + 1) * 128, n2 * 512:(n2 + 1) * 512],
                po[:])

    # ---- attention pipeline: flat stream of k-tile PAIRS ----
    # Each unit = (j, h, p): k-tiles (2p, 2p+1) of chunk j, head h.
    # Scores for both tiles land in one [128,2,512] 2-bank PSUM tile and
    # are exp'd by a single wide ACT instruction; the PV pair trails the
    # score stream by 2 units so the PE never waits on the scalar engine.
    HEAD_ORDER = (1, 0, 3, 2)
    units = []
    for j in range(NQ):
        for h in HEAD_ORDER:
            for p in range(2 * j + 2):
                units.append((j, h, p))

    otps = {}

    DR = mybir.MatmulPerfMode.DoubleRow

    def emit_scores_pair(j, h, p):
        e = h % 2
        qt = qk8[h // 2]
        kt = qk8[2 + h // 2]
        stp2 = pp.tile([128, 2, 512], F32, tag="st", bufs=2)
        pt2 = ptp.tile([128, 2, 512], BF16, tag="pt")
        co0 = None
        for s in range(2):
            i = 2 * p + s
            diag = i >= 4 * j
            co = (i - 4 * j) * 128 if diag else 0
            if s == 0:
                co0 = co
            if diag:
                nc.tensor.matmul(
                    stp2[:, s, co:co + 128],
                    lhsT=kt[:, e, :, i * 128:(i + 1) * 128],
                    rhs=qt[:, e, :, j * 512 + co:j * 512 + co + 128],
                    start=True, stop=False, perf_mode=DR)
                nc.tensor.matmul(
                    stp2[:, s, co:co + 128],
                    lhsT=identb[:], rhs=maskb[:],
                    start=False, stop=True)
                if co < 384:
                    nc.tensor.matmul(
                        stp2[:, s, co + 128:512],
                        lhsT=kt[:, e, :, i * 128:(i + 1) * 128],
                        rhs=qt[:, e, :, j * 512 + co + 128:(j + 1) * 512],
                        start=True, stop=True, perf_mode=DR)
            else:
                nc.tensor.matmul(
                    stp2[:, s, :],
                    lhsT=kt[:, e, :, i * 128:(i + 1) * 128],
                    rhs=qt[:, e, :, j * 512:(j + 1) * 512],
                    start=True, stop=True, perf_mode=DR)
        sf = stp2.rearrange("p a b -> p (a b)")
        pf = pt2.rearrange("p a b -> p (a b)")
        if 2 * p >= 4 * j:
            # diagonal pair: exp exactly the written ranges (the flat span
            # would read never-written PSUM between the two sub-tiles)
            co1 = (2 * p + 1 - 4 * j) * 128
            nc.scalar.activation(pf[:, co0:512], sf[:, co0:512], Exp,
                                 scale=1.0 / 128.0)
            nc.scalar.activation(pf[:, 512 + co1:1024],
                                 sf[:, 512 + co1:1024], Exp,
                                 scale=1.0 / 128.0)
        else:
            nc.scalar.activation(pf[:, 0:1024], sf[:, 0:1024], Exp,
                                 scale=1.0 / 128.0)
        return pt2

    def emit_pv_pair(j, h, p, pt2):
        nkt = 4 * j + 4
        if p == 0:
            otps[(j, h)] = pp.tile([128, 512], F32, tag="ot", bufs=2,
                                   name=f"ot{j}_{h}")
        otp = otps[(j, h)]
        for s in range(2):
            i = 2 * p + s
            co = (i - 4 * j) * 128 if i >= 4 * j else 0
            nc.tensor.matmul(
                otp[:, co:512],
                lhsT=vs[i][:, h, :],
                rhs=pt2[:, s, co:512],
                start=(i == 0), stop=(i == nkt - 1))

    def emit_epilogue(j, h):
        otp = otps.pop((j, h))
        ls = rrp.tile([1, 512], F32, tag="ls")
        nc.vector.tensor_copy(ls[:], otp[64:65, :])
        li = rrp.tile([1, 512], F32, tag="li")
        nc.vector.reciprocal_approx_fast(li[:], ls[:])
        lb = rrp.tile([64, 512], F32, tag="lb")
        nc.gpsimd.partition_broadcast(lb[:], li[:])
        if h % 2 == 0:
            nc.vector.tensor_mul(otj[j][0:64, h // 2, :], otp[0:64, :], lb[:])
        else:
            tm = tmpn.tile([64, 512], BF16, tag="tm")
            nc.vector.tensor_mul(tm[:], otp[0:64, :], lb[:])
            nc.gpsimd.dma_start(otj[j][64:128, h // 2, :], tm[:])

    # ---- filler assignment (unit index -> list of emit fns) ----
    # chunk 0 heads 1,0 carry the second qk wave (heads 2,3 of chunk 0
    # need it); V tiles 4..15 ride chunk-0 heads 3,2 and the start of
    # chunk 1; proj(j) rides the first head of chunk j+1.
    fillers = {}

    def add_fill(uidx, fn):
        fillers.setdefault(uidx, []).append(fn)

    uidx_of = {u: i for i, u in enumerate(units)}
    a2 = [(m, n) for m in (1, 3) for n in range(NQ)]
    for t, (m, n) in enumerate(a2):
        # units (0,1,p=0/1) and (0,0,p=0/1): 2 per unit
        base = uidx_of[(0, HEAD_ORDER[t // 4], 0)]
        add_fill(base + (t // 2) % 2, lambda m=m, n=n: emit_A2_unit(m, n))
    for t, i in enumerate(range(4, 12)):
        base = uidx_of[(0, HEAD_ORDER[2 + t // 4], 0)]
        add_fill(base + (t // 2) % 2, lambda i=i: emit_V_unit(i))
    for t, i in enumerate(range(12, 16)):
        base = uidx_of[(1, HEAD_ORDER[0], 0)]
        add_fill(base + t // 2, lambda i=i: emit_V_unit(i))
    for j in range(NQ - 1):
        # +STAG: chunk j's last epilogue is emitted STAG units into chunk
        # j+1; proj(j) must come after it in program order.
        base = uidx_of[(j + 1, HEAD_ORDER[0], 0)] + 3
        for tbl in range(4):
            add_fill(base + tbl, lambda j=j, tbl=tbl: emit_proj_unit(j, tbl))

    # ---- pre-attention: V tiles 0..3 (needed by chunk 0 PV) ----
    for i in range(4):
        emit_V_unit(i)

    # ---- run the pipeline (PV lags scores by 2 units) ----
    STAG = 3
    pend = {}
    for uidx, (j, h, p) in enumerate(units):
        for fn in fillers.get(uidx, ()):
            fn()
        pend[uidx] = (j, h, p, emit_scores_pair(j, h, p))
        back = uidx - STAG
        if back >= 0:
            bj, bh, bp, bpt = pend.pop(back)
            emit_pv_pair(bj, bh, bp, bpt)
            if bp == 2 * bj + 1:  # chunk-head complete
                emit_epilogue(bj, bh)
    for back in sorted(pend):
        bj, bh, bp, bpt = pend.pop(back)
        emit_pv_pair(bj, bh, bp, bpt)
        if bp == 2 * bj + 1:
            emit_epilogue(bj, bh)
    for tbl in range(4):
        emit_proj_unit(3, tbl)


_NC_CACHE = None


def _get_nc():
    global _NC_CACHE
    if _NC_CACHE is None:
        _NC_CACHE = build_nc()
    return _NC_CACHE


def make_in_maps(x, w_qkv, w_proj):
    x = np.asarray(x, np.float32)
    w_qkv = np.asarray(w_qkv, np.float32)
    w_proj = np.asarray(w_proj, np.float32)
    bf = ml_dtypes.bfloat16
    in_maps = []
    for c in range(8):
        b, g = divmod(c, 4)
        # x4 on q and k lifts them into fp8e4's normal range; the exp
        # activation unscales by 1/(4*4*sqrt(Dh)) = 1/128.
        wq = w_qkv[g * 256:(g + 1) * 256] * 4.0
        wk = w_qkv[C + g * 256:C + (g + 1) * 256] * 4.0
        wv = w_qkv[2 * C + g * 256:2 * C + (g + 1) * 256]
        wqk = np.concatenate([wq, wk], 0)  # [512, C]
        in_maps.append({
            "xt": np.ascontiguousarray(x[b].T).astype(bf),
            "wqkt": np.ascontiguousarray(wqk.T).astype(bf),
            "wvt": np.ascontiguousarray(wv.T).astype(bf),
            "wpt": np.ascontiguousarray(
                w_proj[:, g * 256:(g + 1) * 256].T).astype(bf),
        })
    return in_maps


def combine(results):
    return np.stack(
        [results[4 * b]["p"] + results[4 * b + 1]["p"]
         + results[4 * b + 2]["p"] + results[4 * b + 3]["p"]
         for b in range(2)], 0)


def kernel(x, w_qkv, w_proj):
    nc = _get_nc()
    res = bass_utils.run_bass_kernel_spmd(
        nc, make_in_maps(x, w_qkv, w_proj), core_ids=list(range(8)))
    return combine(res.results)


# revision 18
# speedup vs baseline: 1.2210x; 1.2210x over previous
"""Causal self-attention (B=2, T=2048, C=1024, H=16, Dh=64) on 8 trn2 NeuronCores.

Sharding: 2-way data-parallel over batch x 4-way tensor-parallel over heads.
Core c handles batch b=c//4 and heads 4g..4g+3 where g=c%4; it returns a
partial [T, C] row-projection which the host sums over the 4 head groups.

v2 schedule (vs the 207us baseline):
- DMA-paced start: x and the qkv weights stream on two rings while the
  first qk projection wave runs k-OUTER across 8 PSUM banks, so the PE
  starts ~1.5us in instead of waiting for the full 4MB of x.
- Chunk-major attention: outer loop over q-chunks, then heads, then
  k-tiles. Only one PV accumulator bank is live at a time, leaving PSUM
  room to double-buffer scores, and the output projection (+ HBM
  writeback) for chunk j fires as soon as its 4 heads finish - output DMA
  is spread across the whole attention span instead of a 13us tail.
- No mask matmuls: the causal triangle of each diagonal block is zeroed
  POST-exp by a gpsimd affine_select (idle engine), saving ~20k PE
  columns and 64 weight loads.
- PV lags scores by one k-tile so the scalar-engine exp latency never
  stalls the PE; independent GEMM work (V projection, second qk wave,
  output projection) is interleaved into the attention loops as filler.
- Epilogue per (chunk, head): softmax denominator comes free as a ones
  column in the PV stationary; 1/l via reciprocal_approx_fast (DVE) +
  gpsimd partition broadcast, fused into the PSUM->SBUF normalize mul.
"""

import numpy as np
import ml_dtypes
from contextlib import ExitStack

import concourse.bass as bass
import concourse.tile as tile
from concourse import bacc, mybir, bass_utils

F32 = mybir.dt.float32
BF16 = mybir.dt.bfloat16

T = 2048
C = 1024
HL = 4   # local heads per core
DH = 64
NKT = T // 128   # 16 k-tiles
NQ = T // 512    # 4 q-chunks
NCC = C // 128   # 8 contraction chunks


def _pin_act_table():
    import concourse.bacc as bacc_mod
    from concourse.hw_specs import get_activation_tables as real

    def only_combined(arch):
        t = real(arch)
        name = "natural_log_exp_and_others"
        if name in t:
            return {name: t[name]}
        return t

    bacc_mod.get_activation_tables = only_combined


def build_nc():
    _pin_act_table()
    nc = bacc.Bacc("TRN2", target_bir_lowering=False, debug=False)
    xt_d = nc.dram_tensor("xt", [C, T], BF16, kind="ExternalInput").ap()
    wqkt_d = nc.dram_tensor("wqkt", [C, 512], BF16, kind="ExternalInput").ap()
    wvt_d = nc.dram_tensor("wvt", [C, 256], BF16, kind="ExternalInput").ap()
    wpt_d = nc.dram_tensor("wpt", [256, C], BF16, kind="ExternalInput").ap()
    p_d = nc.dram_tensor("p", [T, C], F32, kind="ExternalOutput").ap()

    with tile.TileContext(nc) as tc:
        with ExitStack() as ctx:
            _body(ctx, tc, xt_d, wqkt_d, wvt_d, wpt_d, p_d)
    nc.compile()
    return nc


def _body(ctx, tc, xt_d, wqkt_d, wvt_d, wpt_d, p_d):
    nc = tc.nc
    Exp = mybir.ActivationFunctionType.Exp

    persist = ctx.enter_context(tc.tile_pool(name="persist", bufs=1))
    ptp = ctx.enter_context(tc.tile_pool(name="ptp", bufs=5))
    rrp = ctx.enter_context(tc.tile_pool(name="rrp", bufs=4))
    tmpn = ctx.enter_context(tc.tile_pool(name="tmpn", bufs=2))
    pout = ctx.enter_context(tc.tile_pool(name="pout", bufs=4))
    # PSUM (8 banks): "st" ring of 2 x [128,2,512] (paired score tiles,
    # 4 banks), "ot" ring of 2 (PV accumulators), "wa" ring of 2 (filler
    # GEMM units: V / second qk wave / projection). 4+2+2 = 8.
    pp = ctx.enter_context(tc.tile_pool(name="pp", bufs=1, space="PSUM"))

    # ---- persistent SBUF tiles ----
    onesf = persist.tile([128, 64], F32, tag="onesf")
    xT = persist.tile([128, NCC, T], BF16, tag="xT")
    wqkT = persist.tile([128, NCC, 512], BF16, tag="wqkT")
    wvT = persist.tile([128, NCC, 256], BF16, tag="wvT")
    wpT = persist.tile([128, 2, C], BF16, tag="wpT")
    qkT = [persist.tile([128, T], BF16, tag=f"qkT{m}", name=f"qkT{m}")
           for m in range(4)]
    vs = [persist.tile([128, HL, 128], BF16, tag=f"vs{i}", name=f"vs{i}")
          for i in range(NKT)]
    otj = [persist.tile([128, 2, 512], BF16, tag=f"otj{j}", name=f"otj{j}")
           for j in range(NQ)]
    identb = persist.tile([128, 128], BF16, tag="identb")
    maskb = persist.tile([128, 128], BF16, tag="maskb")

    nc.gpsimd.memset(onesf[:], 1.0)
    # bf16 identity (for PSUM-accumulate mask adds) and the causal triangle
    # mask: maskb[k, q] = 0 where q >= k else -30000 (additive, pre-exp).
    nc.gpsimd.memset(identb[:], 0.0)
    nc.gpsimd.affine_select(
        out=identb[:], in_=identb[:], compare_op=mybir.AluOpType.not_equal,
        fill=1.0, base=0, channel_multiplier=1, pattern=[[-1, 128]])
    nc.gpsimd.memset(maskb[:], 0.0)
    nc.gpsimd.affine_select(
        out=maskb[:], in_=maskb[:], compare_op=mybir.AluOpType.is_ge,
        fill=-30000.0, base=0, channel_multiplier=-1, pattern=[[1, 128]])
    for i in range(NKT):
        # col 64 of each head strip = softmax denominator ones; 65..127 pad.
        nc.vector.tensor_copy(
            vs[i][:, :, 64:128],
            onesf[:, 0:64].rearrange("p (a b) -> p a b", a=1).to_broadcast(
                (128, HL, 64)))

    # ---- input DMA: x split across sync+scalar rings, weights on gpsimd ----
    for k in range(NCC):
        eng = nc.sync if k % 2 == 0 else nc.scalar
        eng.dma_start(xT[:, k, :], xt_d[k * 128:(k + 1) * 128, :])
    for k in range(NCC):
        nc.gpsimd.dma_start(wqkT[:, k, :], wqkt_d[k * 128:(k + 1) * 128, :])
    for k in range(NCC):
        nc.gpsimd.dma_start(wvT[:, k, :], wvt_d[k * 128:(k + 1) * 128, :])
    for c in range(2):
        nc.gpsimd.dma_start(wpT[:, c, :], wpt_d[c * 128:(c + 1) * 128, :])

    # ---- joint qk wave m=0,2 (heads 0,1): k-outer across all 8 banks,
    # paced by x-chunk arrival so the PE starts ~1.5us in ----
    stA = [pp.tile([128, 2, 512], F32, tag="st", bufs=2, name=f"stA{t}")
           for t in range(2)]
    otA = [pp.tile([128, 512], F32, tag="ot", bufs=2, name=f"otA{t}")
           for t in range(2)]
    waA = [pp.tile([128, 512], F32, tag="wa", bufs=2, name=f"waA{t}")
           for t in range(2)]
    slotA = {}
    for n in range(NQ):
        slotA[(0, n)] = stA[n // 2][:, n % 2, :]
    slotA[(2, 0)] = otA[0][:]
    slotA[(2, 1)] = otA[1][:]
    slotA[(2, 2)] = waA[0][:]
    slotA[(2, 3)] = waA[1][:]
    for k in range(NCC):
        for m in (0, 2):
            for n in range(NQ):
                nc.tensor.matmul(
                    slotA[(m, n)],
                    lhsT=wqkT[:, k, m * 128:(m + 1) * 128],
                    rhs=xT[:, k, n * 512:(n + 1) * 512],
                    start=(k == 0), stop=(k == NCC - 1))
    for m in (0, 2):
        for n in range(NQ):
            if n % 2 == 0:
                nc.scalar.copy(qkT[m][:, n * 512:(n + 1) * 512], slotA[(m, n)])
            else:
                nc.vector.tensor_copy(qkT[m][:, n * 512:(n + 1) * 512],
                                      slotA[(m, n)])

    # ---- filler GEMM units (one "wa" bank each) ----
    def emit_A2_unit(m, n):  # one (m,n) block of the second qk wave
        ps = pp.tile([128, 512], F32, tag="wa", bufs=2)
        for k in range(NCC):
            nc.tensor.matmul(
                ps[:],
                lhsT=wqkT[:, k, m * 128:(m + 1) * 128],
                rhs=xT[:, k, n * 512:(n + 1) * 512],
                start=(k == 0), stop=(k == NCC - 1))
        nc.vector.tensor_copy(qkT[m][:, n * 512:(n + 1) * 512], ps[:])

    def emit_V_unit(i):  # v tile i
        ps = pp.tile([128, 256], F32, tag="wa", bufs=2)
        for k in range(NCC):
            nc.tensor.matmul(
                ps[:],
                lhsT=xT[:, k, i * 128:(i + 1) * 128],
                rhs=wvT[:, k, :],
                start=(k == 0), stop=(k == NCC - 1))
        nc.vector.tensor_copy(
            vs[i][:, :, 0:64], ps[:].rearrange("p (h d) -> p h d", h=HL))

    def emit_proj_unit(j, tbl):  # one 128-row block of chunk j's projection
        tb = 4 * j + tbl
        for n2 in range(2):
            ps = pp.tile([128, 512], F32, tag="wa", bufs=2)
            for c in range(2):
                nc.tensor.matmul(
                    ps[:],
                    lhsT=otj[j][:, c, tbl * 128:(tbl + 1) * 128],
                    rhs=wpT[:, c, n2 * 512:(n2 + 1) * 512],
                    start=(c == 0), stop=(c == 1))
            po = pout.tile([128, 512], F32, tag="po")
            nc.vector.tensor_copy(po[:], ps[:])
            nc.sync.dma_start(
                p_d[tb * 128:(tb + 1) * 128, n2 * 512:(n2 + 1) * 512],
                po[:])

    # ---- attention pipeline: flat stream of k-tile PAIRS ----
    # Each unit = (j, h, p): k-tiles (2p, 2p+1) of chunk j, head h.
    # Scores for both tiles land in one [128,2,512] 2-bank PSUM tile and
    # are exp'd by a single wide ACT instruction; the PV pair trails the
    # score stream by 2 units so the PE never waits on the scalar engine.
    HEAD_ORDER = (1, 0, 3, 2)
    units = []
    for j in range(NQ):
        for h in HEAD_ORDER:
            for p in range(2 * j + 2):
                units.append((j, h, p))

    otps = {}

    def emit_scores_pair(j, h, p):
        part = (h % 2) * 64
        qt = qkT[h // 2]
        kt = qkT[2 + h // 2]
        stp2 = pp.tile([128, 2, 512], F32, tag="st", bufs=2)
        pt2 = ptp.tile([128, 2, 512], BF16, tag="pt")
        co0 = None
        for s in range(2):
            i = 2 * p + s
            diag = i >= 4 * j
            co = (i - 4 * j) * 128 if diag else 0
            if s == 0:
                co0 = co
            if diag:
                nc.tensor.matmul(
                    stp2[:, s, co:co + 128],
                    lhsT=kt[part:part + 64, i * 128:(i + 1) * 128],
                    rhs=qt[part:part + 64,
                           j * 512 + co:j * 512 + co + 128],
                    start=True, stop=False)
                nc.tensor.matmul(
                    stp2[:, s, co:co + 128],
                    lhsT=identb[:], rhs=maskb[:],
                    start=False, stop=True)
                if co < 384:
                    nc.tensor.matmul(
                        stp2[:, s, co + 128:512],
                        lhsT=kt[part:part + 64, i * 128:(i + 1) * 128],
                        rhs=qt[part:part + 64,
                               j * 512 + co + 128:(j + 1) * 512],
                        start=True, stop=True)
            else:
                nc.tensor.matmul(
                    stp2[:, s, :],
                    lhsT=kt[part:part + 64, i * 128:(i + 1) * 128],
                    rhs=qt[part:part + 64, j * 512:(j + 1) * 512],
                    start=True, stop=True)
        sf = stp2.rearrange("p a b -> p (a b)")
        pf = pt2.rearrange("p a b -> p (a b)")
        if 2 * p >= 4 * j:
            # diagonal pair: exp exactly the written ranges (the flat span
            # would read never-written PSUM between the two sub-tiles)
            co1 = (2 * p + 1 - 4 * j) * 128
            nc.scalar.activation(pf[:, co0:512], sf[:, co0:512], Exp)
            nc.scalar.activation(pf[:, 512 + co1:1024],
                                 sf[:, 512 + co1:1024], Exp)
        else:
            nc.scalar.activation(pf[:, 0:1024], sf[:, 0:1024], Exp)
        return pt2

    def emit_pv_pair(j, h, p, pt2):
        nkt = 4 * j + 4
        if p == 0:
            otps[(j, h)] = pp.tile([128, 512], F32, tag="ot", bufs=2,
                                   name=f"ot{j}_{h}")
        otp = otps[(j, h)]
        for s in range(2):
            i = 2 * p + s
            co = (i - 4 * j) * 128 if i >= 4 * j else 0
            nc.tensor.matmul(
                otp[:, co:512],
                lhsT=vs[i][:, h, :],
                rhs=pt2[:, s, co:512],
                start=(i == 0), stop=(i == nkt - 1))

    def emit_epilogue(j, h):
        otp = otps.pop((j, h))
        ls = rrp.tile([1, 512], F32, tag="ls")
        nc.vector.tensor_copy(ls[:], otp[64:65, :])
        li = rrp.tile([1, 512], F32, tag="li")
        nc.vector.reciprocal_approx_fast(li[:], ls[:])
        lb = rrp.tile([64, 512], F32, tag="lb")
        nc.gpsimd.partition_broadcast(lb[:], li[:])
        if h % 2 == 0:
            nc.vector.tensor_mul(otj[j][0:64, h // 2, :], otp[0:64, :], lb[:])
        else:
            tm = tmpn.tile([64, 512], BF16, tag="tm")
            nc.vector.tensor_mul(tm[:], otp[0:64, :], lb[:])
            nc.gpsimd.dma_start(otj[j][64:128, h // 2, :], tm[:])

    # ---- filler assignment (unit index -> list of emit fns) ----
    # chunk 0 heads 1,0 carry the second qk wave (heads 2,3 of chunk 0
    # need it); V tiles 4..15 ride chunk-0 heads 3,2 and the start of
    # chunk 1; proj(j) rides the first head of chunk j+1.
    fillers = {}

    def add_fill(uidx, fn):
        fillers.setdefault(uidx, []).append(fn)

    uidx_of = {u: i for i, u in enumerate(units)}
    a2 = [(m, n) for m in (1, 3) for n in range(NQ)]
    for t, (m, n) in enumerate(a2):
        # units (0,1,p=0/1) and (0,0,p=0/1): 2 per unit
        base = uidx_of[(0, HEAD_ORDER[t // 4], 0)]
        add_fill(base + (t // 2) % 2, lambda m=m, n=n: emit_A2_unit(m, n))
    for t, i in enumerate(range(4, 12)):
        base = uidx_of[(0, HEAD_ORDER[2 + t // 4], 0)]
        add_fill(base + (t // 2) % 2, lambda i=i: emit_V_unit(i))
    for t, i in enumerate(range(12, 16)):
        base = uidx_of[(1, HEAD_ORDER[0], 0)]
        add_fill(base + t // 2, lambda i=i: emit_V_unit(i))
    for j in range(NQ - 1):
        # +STAG: chunk j's last epilogue is emitted STAG units into chunk
        # j+1; proj(j) must come after it in program order.
        base = uidx_of[(j + 1, HEAD_ORDER[0], 0)] + 3
        for tbl in range(4):
            add_fill(base + tbl, lambda j=j, tbl=tbl: emit_proj_unit(j, tbl))

    # ---- pre-attention: V tiles 0..3 (needed by chunk 0 PV) ----
    for i in range(4):
        emit_V_unit(i)

    # ---- run the pipeline (PV lags scores by 2 units) ----
    STAG = 3
    pend = {}
    for uidx, (j, h, p) in enumerate(units):
        for fn in fillers.get(uidx, ()):
            fn()
        pend[uidx] = (j, h, p, emit_scores_pair(j, h, p))
        back = uidx - STAG
        if back >= 0:
            bj, bh, bp, bpt = pend.pop(back)
            emit_pv_pair(bj, bh, bp, bpt)
            if bp == 2 * bj + 1:  # chunk-head complete
                emit_epilogue(bj, bh)
    for back in sorted(pend):
        bj, bh, bp, bpt = pend.pop(back)
        emit_pv_pair(bj, bh, bp, bpt)
        if bp == 2 * bj + 1:
            emit_epilogue(bj, bh)
    for tbl in range(4):
        emit_proj_unit(3, tbl)


_NC_CACHE = None


def _get_nc():
    global _NC_CACHE
    if _NC_CACHE is None:
        _NC_CACHE = build_nc()
    return _NC_CACHE


def make_in_maps(x, w_qkv, w_proj):
    x = np.asarray(x, np.float32)
    w_qkv = np.asarray(w_qkv, np.float32)
    w_proj = np.asarray(w_proj, np.float32)
    bf = ml_dtypes.bfloat16
    in_maps = []
    for c in range(8):
        b, g = divmod(c, 4)
        wq = w_qkv[g * 256:(g + 1) * 256] * 0.125  # fold 1/sqrt(Dh)
        wk = w_qkv[C + g * 256:C + (g + 1) * 256]
        wv = w_qkv[2 * C + g * 256:2 * C + (g + 1) * 256]
        wqk = np.concatenate([wq, wk], 0)  # [512, C]
        in_maps.append({
            "xt": np.ascontiguousarray(x[b].T).astype(bf),
            "wqkt": np.ascontiguousarray(wqk.T).astype(bf),
            "wvt": np.ascontiguousarray(wv.T).astype(bf),
            "wpt": np.ascontiguousarray(
                w_proj[:, g * 256:(g + 1) * 256].T).astype(bf),
        })
    return in_maps


def combine(results):
    return np.stack(
        [results[4 * b]["p"] + results[4 * b + 1]["p"]
         + results[4 * b + 2]["p"] + results[4 * b + 3]["p"]
         for b in range(2)], 0)


def kernel(x, w_qkv, w_proj):
    nc = _get_nc()
    res = bass_utils.run_bass_kernel_spmd(
        nc, make_in_maps(x, w_qkv, w_proj), core_ids=list(range(8)))
    return combine(res.results)
